# revision 26
# baseline (speedup 1.0000x reference)
"""Causal self-attention (B=2, T=2048, C=1024, H=16, D=64) on 8 trn2 NeuronCores.

Sharding: batch x head-group. Core c handles batch b = c//4 and head group
g = c%4 (4 heads = 256 channels). All-bf16 data path (fp32 PSUM accumulate).

Per core:
  - warmup matmul burst at t=0 (overlapping input DMA) so the PE HAM clock
    gate flips to 2.4 GHz before real work, and stays there
  - qkv projection for its 4 heads (Q^T/K^T in [d, t] layout, V in [t, d]);
    x^T DMA'd in four 1MB T-slices, weights in single DMAs
  - causal flash attention (scores K-major as S^T, exp batched across both
    heads of a pair in one ACT call from a 2-bank PSUM tile, diagonal
    blocks trimmed, row-sums via a ones-column appended to V)
  - AllGather of Y^T across all 8 cores (the 8-rank on-chip path is much
    faster than a 4-rank ring), split per (pair, T-half) = 4 collectives
    in bf16, pipelined into attention; proj weight rows for the foreign
    batch are staged as zeros so one SPMD program works for both batches
  - output projection in transposed layout (out^T[oc, t], N=512 moving),
    2-phase accumulation interleaved into pair-1 attention

Host gather: per-core out^T [256, T] -> transpose into [B, T, C] slices.
"""

import os
import numpy as np
import ml_dtypes

import concourse.bass as bass
import concourse.bacc as bacc
import concourse.mybir as mybir
import concourse.tile as tile
from concourse import bass_utils
from concourse.bass import ds, ts
from concourse.bass_interp import get_hw_module

P = 128
B, T, C = 2, 2048, 1024
NH, D = 16, 64
NC = 8          # cores
NG = 4          # head groups (cores per batch)
HL = NH // NG   # heads per core = 4
DL = HL * D     # local channels = 256
NQ = 512        # query tile
F32 = mybir.dt.float32
BF16 = mybir.dt.bfloat16
NPBF16 = ml_dtypes.bfloat16

AG_WORLD = 4    # 4: per-batch AllGather groups; 8: one 8-rank AllGather
NSEG = 2 * NG if AG_WORLD == 8 else NG  # proj row-segs per pair
N_WARMUP = 28   # dummy matmuls to warm the HAM clock gate


def _build_body(ctx, tc, io):
    nc = tc.nc
    xt, wq, wk, wv, bq, bk, bv, wp, bp, tri, out, ytl, ytf = io
    mm = nc.tensor.matmul

    pers = ctx.enter_context(tc.tile_pool(name="pers", bufs=1))
    psum = ctx.enter_context(tc.tile_pool(name="psum", bufs=1, space="PSUM"))
    pp = ctx.enter_context(tc.tile_pool(name="pp", bufs=3))
    nrm = ctx.enter_context(tc.tile_pool(name="nrm", bufs=4))
    po = ctx.enter_context(tc.tile_pool(name="po", bufs=4))
    yf = ctx.enter_context(tc.tile_pool(name="yf", bufs=2))

    # ---- HAM warmup: dense dummy matmuls while input DMAs stream in ----
    dummy = pers.tile([P, NQ], BF16)
    nc.vector.memset(dummy[:], 0.0)
    for _ in range(N_WARMUP):
        wps = psum.tile([P, NQ], F32, tag="gemm", name="warm_ps", bufs=2)
        mm(wps[:], dummy[:, 0:P], dummy[:], start=True, stop=True)

    tri_sb = pers.tile([P, P], BF16)
    qt_sb = pers.tile([P, 2, T], BF16)   # pair j; head 2j+1 on partitions 64..127
    kt_sb = pers.tile([P, 2, T], BF16)
    # [l_part, l_chunk, head, d | 64x ones]: PV with this stationary yields
    # O on partitions 0..63 and the softmax row-sum REPLICATED on 64..127,
    # so normalization needs no cross-partition broadcast at all.
    v_sb = pers.tile([P, T // P, HL, 2 * D], BF16)
    nc.vector.memset(v_sb[:], 1.0)  # cols D..2D stay 1; 0..D overwritten
    # yth[pair]: rows 0..63 head 2p, rows 64..127 head 2p+1 (AG payload layout)
    yth = [pers.tile([P, T], BF16, tag=f"yth{p}", name=f"yth{p}") for p in range(2)]

    xt_sb = pers.tile([P, C // P, T], BF16)
    wq_sb = pers.tile([P, C // P, DL], BF16)
    wk_sb = pers.tile([P, C // P, DL], BF16)
    wv_sb = pers.tile([P, C // P, DL], BF16)
    wp_sb = pers.tile([P, 2 * NSEG, DL], BF16)  # seg s = NSEG*p + r
    acc = pers.tile([P, 2, T], BF16)            # proj phase-A accumulator (out^T)

    bqp = pers.tile([P, 2], F32)
    bkp = pers.tile([P, 2], F32)
    bv_row = pers.tile([1, DL], F32)
    bv_bc = pers.tile([P, DL], F32)
    bpp = pers.tile([P, 2], F32)

    # ---- input DMAs: big transfers, ordered so tt=0 matmuls start early ----
    nc.sync.dma_start(wk_sb[:], wk.rearrange("(c p) n -> p c n", p=P))
    nc.sync.dma_start(wq_sb[:], wq.rearrange("(c p) n -> p c n", p=P))
    for tt in range(T // NQ):
        nc.sync.dma_start(
            xt_sb[:, :, ts(tt, NQ)],
            xt[:, ts(tt, NQ)].rearrange("(c p) t -> p c t", p=P),
        )
        if tt == 0:
            nc.sync.dma_start(bqp[:], bq.rearrange("(j p) -> p j", p=P))
            nc.sync.dma_start(bkp[:], bk.rearrange("(j p) -> p j", p=P))
            nc.sync.dma_start(bv_row[:], bv[None, :])
            nc.gpsimd.partition_broadcast(bv_bc[:], bv_row[:])
            nc.sync.dma_start(tri_sb[:], tri)
    nc.sync.dma_start(wv_sb[:], wv.rearrange("(c p) n -> p c n", p=P))
    nc.sync.dma_start(wp_sb[:], wp.rearrange("(s p) n -> p s n", p=P))
    nc.sync.dma_start(bpp[:], bp.rearrange("(o p) -> p o", p=P))

    def qk_tile(w_sb, b_sb, dst, j, tt):
        ps = psum.tile([P, NQ], F32, tag="gemm", name="qk_ps", bufs=2)
        for cc in range(C // P):
            mm(
                ps[:],
                w_sb[:, cc, ts(j, P)],
                xt_sb[:, cc, ts(tt, NQ)],
                start=(cc == 0),
                stop=(cc == C // P - 1),
            )
        nc.vector.tensor_scalar_add(dst[:, j, ts(tt, NQ)], ps[:], b_sb[:, j : j + 1])

    def v_tile(tt):
        ps = psum.tile([P, DL], F32, tag="gemm", name="v_ps", bufs=2)
        for cc in range(C // P):
            mm(
                ps[:],
                xt_sb[:, cc, ts(tt, P)],
                wv_sb[:, cc, :],
                start=(cc == 0),
                stop=(cc == C // P - 1),
            )
        nc.vector.tensor_add(
            v_sb[:, tt, :, 0:D],
            ps[:].rearrange("p (h d) -> p h d", h=HL),
            bv_bc[:].rearrange("p (h d) -> p h d", h=HL),
        )  # cols D..2D keep their memset 1.0 (row-sum columns)

    def attn_qtile(pair, qt):
        q0 = NQ * qt
        nl = q0 // P + NQ // P  # l-chunks for causal coverage
        last_mm = [None]  # last PV matmul of this qtile (for sched ordering)
        o_ps = [
            psum.tile([P, NQ], F32, tag=f"o{hi}", name=f"o_ps{hi}", bufs=1)
            for hi in range(2)
        ]

        def s_stage(lc):
            w0 = max(P * lc - q0, 0)
            s2 = psum.tile([P, 2, NQ], F32, tag="s", name="s2", bufs=2)
            for hi in range(2):
                mm(
                    s2[:, hi, w0:NQ],
                    kt_sb[64 * hi : 64 * hi + 64, pair, ts(lc, P)],
                    qt_sb[64 * hi : 64 * hi + 64, pair, ds(q0 + w0, NQ - w0)],
                    start=True,
                    stop=True,
                    tile_position=(64 * hi, 0),
                )
            return s2

        def pv_stage(lc, s2):
            off = P * lc - q0
            w0 = max(off, 0)
            pt = pp.tile([P, 2, NQ], BF16, tag="p", name="pt")
            nc.scalar.activation(
                pt[:, :, w0:NQ],
                s2[:, :, w0:NQ],
                mybir.ActivationFunctionType.Exp,
                bias=0.0,
                scale=1.0 / np.sqrt(D),
            )
            if off >= 0:
                for hi in range(2):
                    nc.vector.tensor_mul(
                        pt[:, hi, off : off + P],
                        pt[:, hi, off : off + P],
                        tri_sb[:],
                    )
            for hi in range(2):
                last_mm[0] = mm(
                    o_ps[hi][:, w0:NQ],
                    v_sb[:, lc, 2 * pair + hi, :],
                    pt[:, hi, w0:NQ],
                    start=(lc == 0),
                    stop=(lc == nl - 1),
                )  # partitions 0..63 = O, 64..127 = replicated row-sums

        # software pipeline: keep one S stage ahead of exp/PV
        prev = s_stage(0)
        for lc in range(1, nl):
            cur = s_stage(lc)
            pv_stage(lc - 1, prev)
            prev = cur
        pv_stage(nl - 1, prev)

        for hi in range(2):
            sums_sb = nrm.tile([D, NQ], F32, tag="sums")
            nc.vector.tensor_copy(sums_sb[:], o_ps[hi][D : 2 * D, :])
            rcp = nrm.tile([D, NQ], F32, tag="rcp")
            nc.vector.reciprocal_approx_fast(rcp[:], sums_sb[:])
            nc.vector.tensor_mul(
                yth[pair][64 * hi : 64 * hi + 64, ds(q0, NQ)],
                o_ps[hi][0:D, :],
                rcp[:],
            )
        return last_mm[0]

    if AG_WORLD == 8:
        replica_groups = [list(range(NC))]
    else:
        replica_groups = [[0, 1, 2, 3], [4, 5, 6, 7]]

    def ship(pair, half):
        # DMA this (pair, T-half) of Y^T to HBM and AllGather it.
        # Trigger on the (mostly idle) gpsimd queue so it doesn't wait
        # behind bulk input DMAs on the sync queue.
        nc.sync.dma_start(ytl[pair][half][:], yth[pair][:, ts(half, T // 2)])
        nc.gpsimd.collective_compute(
            "AllGather",
            mybir.AluOpType.bypass,
            replica_groups=replica_groups,
            ins=[ytl[pair][half][:]],
            outs=[ytf[pair][half][:]],
        )

    def proj_half(pair, half, not_before=None):
        # one pair's contribution to out^T for one T-half. `not_before`
        # orders the first matmul after the given instruction so the PE
        # stream never head-of-line blocks on the (uncosted) AllGather.
        y = yf.tile([P, NSEG, T // 2], BF16, tag=f"y{pair}", name=f"y{pair}")
        nc.sync.dma_start(
            y[:], ytf[pair][half].rearrange("(g p) t -> p g t", p=P)
        )
        first = [True]
        for oc in range(2):
            for s in range(2):  # 512-col subtiles of the half
                t0 = half * (T // 2) + s * NQ
                ps = psum.tile([P, NQ], F32, tag="gemm", name="pr_ps", bufs=2)
                for g in range(NSEG):
                    inst = mm(
                        ps[:],
                        wp_sb[:, NSEG * pair + g, ts(oc, P)],
                        y[:, g, ts(s, NQ)],
                        start=(g == 0),
                        stop=(g == NSEG - 1),
                    )
                    if first[0] and not_before is not None:
                        bass._add_dep_helper(
                            inst.ins,
                            not_before.ins,
                            sync=False,
                            reason="proj after attention (AG not costed)",
                        )
                    first[0] = False
                if pair == 0:
                    nc.vector.tensor_scalar_add(
                        acc[:, oc, ds(t0, NQ)], ps[:], bpp[:, oc : oc + 1]
                    )
                else:
                    ot = po.tile([P, NQ], F32, tag="ot")
                    nc.vector.tensor_add(ot[:], ps[:], acc[:, oc, ds(t0, NQ)])
                    nc.sync.dma_start(out[ts(oc, P), ds(t0, NQ)], ot[:])

    # ---------------- program ----------------
    # Phase 1: all of QKV as one dense matmul block (PE stays HAM-warm).
    for tt in range(T // NQ):
        qk_tile(wk_sb, bkp, kt_sb, 0, tt)
        qk_tile(wq_sb, bqp, qt_sb, 0, tt)
    for tt in range(0, 4):
        v_tile(tt)
    for tt in range(T // NQ):
        qk_tile(wk_sb, bkp, kt_sb, 1, tt)
        qk_tile(wq_sb, bqp, qt_sb, 1, tt)
    for tt in range(4, T // P):
        v_tile(tt)

    # Phase 2: attention (ACT-bound; ships pipelined out via collectives).
    marks = {}
    for qt in range(T // NQ):
        marks[(0, qt)] = attn_qtile(0, qt)
        if qt == 1:
            ship(0, 0)
    ship(0, 1)
    for qt in range(T // NQ):
        marks[(1, qt)] = attn_qtile(1, qt)
        if qt == 1:
            ship(1, 0)
    ship(1, 1)

    # Phase 3: output projection. Pair-0 contributions are ordered after
    # attn(1,0) (their AGs are long done by then) so they fill PE idle
    # during late pair-1 attention and the final AG; pair-1 after the end.
    proj_half(0, 0, not_before=marks[(1, 0)])
    proj_half(0, 1, not_before=marks[(1, 1)])
    proj_half(1, 0, not_before=marks[(1, 3)])
    proj_half(1, 1, not_before=marks[(1, 3)])


def build_program():
    nc = bacc.Bacc(
        "TRN2",
        target_bir_lowering=False,
        debug=False,
        enable_asserts=False,
        num_devices=NC,
    )
    xt = nc.dram_tensor("xt", [C, T], BF16, kind="ExternalInput").ap()
    wq = nc.dram_tensor("wq", [C, DL], BF16, kind="ExternalInput").ap()
    wk = nc.dram_tensor("wk", [C, DL], BF16, kind="ExternalInput").ap()
    wv = nc.dram_tensor("wv", [C, DL], BF16, kind="ExternalInput").ap()
    bq = nc.dram_tensor("bq", [DL], F32, kind="ExternalInput").ap()
    bk = nc.dram_tensor("bk", [DL], F32, kind="ExternalInput").ap()
    bv = nc.dram_tensor("bv", [DL], F32, kind="ExternalInput").ap()
    wp = nc.dram_tensor("wp", [2 * NSEG * P, DL], BF16, kind="ExternalInput").ap()
    bp = nc.dram_tensor("bp", [DL], F32, kind="ExternalInput").ap()
    tri = nc.dram_tensor("tri", [P, P], BF16, kind="ExternalInput").ap()
    out = nc.dram_tensor("out", [DL, T], F32, kind="ExternalOutput").ap()
    ytl = [
        [
            nc.dram_tensor(f"ytl{p}_{h}", [P, T // 2], BF16, kind="Internal").ap()
            for h in range(2)
        ]
        for p in range(2)
    ]
    ytf = [
        [
            nc.dram_tensor(
                f"ytf{p}_{h}",
                [AG_WORLD * P, T // 2],
                BF16,
                kind="Internal",
                addr_space="Shared" if AG_WORLD == 8 else "Local",
            ).ap()
            for h in range(2)
        ]
        for p in range(2)
    ]
    io = (xt, wq, wk, wv, bq, bk, bv, wp, bp, tri, out, ytl, ytf)
    with tile.TileContext(nc) as tc:
        import contextlib

        with contextlib.ExitStack() as ctx:
            _build_body(ctx, tc, io)
    nc.compile()
    return nc


def _stage_wp(W_proj, b, g):
    """wp rows permuted to match ytf row order (rank-major, per pair).

    With AG_WORLD == 8, ytf[p][half] row block r (128 rows) comes from core
    r, carrying batch r//4, group r%4, heads (4*(r%4) + 2p + {0,1}). Blocks
    of the foreign batch get zero weights so one SPMD program serves both
    batches. With AG_WORLD == 4, blocks are the 4 same-batch ranks.
    """
    cols = slice(DL * g, DL * (g + 1))
    Wl = W_proj[:, cols]  # [C, DL]
    segs = []
    for p in range(2):
        if AG_WORLD == 8:
            for r in range(NC):
                if r // NG == b:
                    h0 = 4 * (r % NG) + 2 * p
                    segs.append(Wl[64 * h0 : 64 * h0 + 128, :])
                else:
                    segs.append(np.zeros((P, DL), np.float32))
        else:
            for r in range(NG):
                h0 = 4 * r + 2 * p
                segs.append(Wl[64 * h0 : 64 * h0 + 128, :])
    return np.ascontiguousarray(np.concatenate(segs, axis=0)).astype(NPBF16)


def make_in_maps(x, W_attn, b_attn, W_proj, b_proj):
    # scores are computed transposed (S^T[l, q]); position (l', q'') in a
    # diagonal 128x128 block is causally valid iff q'' >= l' -> upper-tri mask
    tri_np = np.triu(np.ones((P, P), dtype=np.float32)).astype(NPBF16)
    x = np.asarray(x, dtype=np.float32)
    W_attn = np.asarray(W_attn, dtype=np.float32)
    b_attn = np.asarray(b_attn, dtype=np.float32)
    W_proj = np.asarray(W_proj, dtype=np.float32)
    b_proj = np.asarray(b_proj, dtype=np.float32)
    in_maps = []
    for c in range(NC):
        b, g = divmod(c, NG)
        cols = slice(DL * g, DL * (g + 1))
        in_maps.append(
            {
                "xt": np.ascontiguousarray(x[b].T).astype(NPBF16),
                "wq": np.ascontiguousarray(W_attn[:, cols]).astype(NPBF16),
                "wk": np.ascontiguousarray(W_attn[:, C:][:, cols]).astype(NPBF16),
                "wv": np.ascontiguousarray(W_attn[:, 2 * C :][:, cols]).astype(
                    NPBF16
                ),
                "bq": np.ascontiguousarray(b_attn[cols]),
                "bk": np.ascontiguousarray(b_attn[C:][cols]),
                "bv": np.ascontiguousarray(b_attn[2 * C :][cols]),
                "wp": _stage_wp(W_proj, b, g),
                "bp": np.ascontiguousarray(b_proj[cols]),
                "tri": tri_np,
            }
        )
    return in_maps


_NC_CACHE = {}


def _install_ntff_hook():
    """Recreate the missing antenv.axon_hooks module so
    run_bass_kernel_spmd(trace=True) can capture NTFF profiles under axon."""
    import sys
    import types

    if "antenv.axon_hooks" in sys.modules:
        return True
    try:
        from trn_agent_boot.trn_boot import _ntff_profile_via_ctypes

        hook = _ntff_profile_via_ctypes("/opt/axon/libaxon_pjrt.so")
        if hook is None:
            return False
        mod = types.ModuleType("antenv.axon_hooks")
        mod.get_axon_ntff_profile_hook = lambda: hook
        mod.set_axon_ntff_profile_hook = lambda h: None
        sys.modules["antenv.axon_hooks"] = mod
        import antenv

        antenv.axon_hooks = mod
        # keep trace artifacts local (no fish bucket in this container)
        bass_utils.upload_artifacts = lambda tmpdir: tmpdir
        return True
    except Exception:
        return False


def _get_program():
    if "nc" not in _NC_CACHE:
        nc = build_program()
        nc.m = get_hw_module(nc.m)
        _NC_CACHE["nc"] = nc
    return _NC_CACHE["nc"]


def kernel(x, W_attn, b_attn, W_proj, b_proj):
    nc = _get_program()
    in_maps = make_in_maps(x, W_attn, b_attn, W_proj, b_proj)
    trace = bool(int(os.environ.get("KERNEL_TRACE", "0")))
    if trace:
        trace = _install_ntff_hook()
    res = bass_utils.run_bass_kernel_spmd(
        nc,
        in_maps,
        core_ids=list(range(NC)),
        trace=trace,
        trace_cores=list(range(NC)) if trace else None,
    )
    if trace:
        _NC_CACHE["last_results"] = res
        if res.exec_time_ns is not None:
            print(f"HW exec time: {res.exec_time_ns} ns")
            if res.instructions_and_trace is not None:
                print(f"trace: {res.instructions_and_trace[1]}")
    out = np.empty((B, T, C), dtype=np.float32)
    for c in range(NC):
        b, g = divmod(c, NG)
        out[b, :, DL * g : DL * (g + 1)] = res.results[c]["out"].T
    return out


# revision 34
# speedup vs baseline: 1.0582x; 1.0582x over previous
"""Causal self-attention (B=2, T=2048, C=1024, H=16, D=64) on 8 trn2 NeuronCores.

Sharding: batch x head-group. Core c handles batch b = c//4 and head group
g = c%4 (4 heads = 256 channels). All-bf16 data path (fp32 PSUM accumulate).

Per core:
  - warmup matmul burst at t=0 (overlapping input DMA) so the PE HAM clock
    gate flips to 2.4 GHz before real work, and stays there
  - qkv projection for its 4 heads (Q^T/K^T in [d, t] layout, V in [t, d]);
    x^T DMA'd in four 1MB T-slices, weights in single DMAs
  - causal flash attention (scores K-major as S^T, exp batched across both
    heads of a pair in one ACT call from a 2-bank PSUM tile, diagonal
    blocks trimmed, row-sums via a ones-column appended to V)
  - AllGather of Y^T across all 8 cores (the 8-rank on-chip path is much
    faster than a 4-rank ring), split per (pair, T-half) = 4 collectives
    in bf16, pipelined into attention; proj weight rows for the foreign
    batch are staged as zeros so one SPMD program works for both batches
  - output projection in transposed layout (out^T[oc, t], N=512 moving),
    2-phase accumulation interleaved into pair-1 attention

Host gather: per-core out^T [256, T] -> transpose into [B, T, C] slices.
"""

import os
import numpy as np
import ml_dtypes

import concourse.bass as bass
import concourse.bacc as bacc
import concourse.mybir as mybir
import concourse.tile as tile
from concourse import bass_utils
from concourse.bass import ds, ts
from concourse.bass_interp import get_hw_module

P = 128
B, T, C = 2, 2048, 1024
NH, D = 16, 64
NC = 8          # cores
NG = 4          # head groups (cores per batch)
HL = NH // NG   # heads per core = 4
DL = HL * D     # local channels = 256
NQ = 512        # query tile
F32 = mybir.dt.float32
BF16 = mybir.dt.bfloat16
NPBF16 = ml_dtypes.bfloat16

AG_WORLD = 4    # 4: per-batch AllGather groups; 8: one 8-rank AllGather
NSEG = 2 * NG if AG_WORLD == 8 else NG  # proj row-segs per pair
N_WARMUP = 28   # dummy matmuls to warm the HAM clock gate
# Y^T column chunks shipped per pair (start, width). Pair 1's tail is
# quarter-sized so the last (floor-bound) AllGather starts one qtile early.
SHIP_PLAN = {
    0: [(0, T // 2), (T // 2, T // 2)],
    1: [(0, T // 2), (T // 2, T // 4), (3 * T // 4, T // 4)],
}


def _build_body(ctx, tc, io):
    nc = tc.nc
    xt, wq, wk, wv, bq, bk, bv, wp, bp, tri, out, ytl, ytf = io
    mm = nc.tensor.matmul

    pers = ctx.enter_context(tc.tile_pool(name="pers", bufs=1))
    psum = ctx.enter_context(tc.tile_pool(name="psum", bufs=1, space="PSUM"))
    pp = ctx.enter_context(tc.tile_pool(name="pp", bufs=3))
    nrm = ctx.enter_context(tc.tile_pool(name="nrm", bufs=4))
    po = ctx.enter_context(tc.tile_pool(name="po", bufs=4))
    yf = ctx.enter_context(tc.tile_pool(name="yf", bufs=2))

    # ---- HAM warmup: dense dummy matmuls while input DMAs stream in ----
    dummy = pers.tile([P, NQ], BF16)
    nc.vector.memset(dummy[:], 0.0)
    for _ in range(N_WARMUP):
        wps = psum.tile([P, NQ], F32, tag="gemm", name="warm_ps", bufs=2)
        mm(wps[:], dummy[:, 0:P], dummy[:], start=True, stop=True)

    tri_sb = pers.tile([P, P], BF16)
    qt_sb = pers.tile([P, 2, T], BF16)   # pair j; head 2j+1 on partitions 64..127
    kt_sb = pers.tile([P, 2, T], BF16)
    # [l_part, l_chunk, head, d | 64x ones]: PV with this stationary yields
    # O on partitions 0..63 and the softmax row-sum REPLICATED on 64..127,
    # so normalization needs no cross-partition broadcast at all.
    v_sb = pers.tile([P, T // P, HL, 2 * D], BF16)
    nc.vector.memset(v_sb[:], 1.0)  # cols D..2D stay 1; 0..D overwritten
    # yth[pair]: rows 0..63 head 2p, rows 64..127 head 2p+1 (AG payload layout)
    yth = [pers.tile([P, T], BF16, tag=f"yth{p}", name=f"yth{p}") for p in range(2)]

    xt_sb = pers.tile([P, C // P, T], BF16)
    wq_sb = pers.tile([P, C // P, DL], BF16)
    wk_sb = pers.tile([P, C // P, DL], BF16)
    wv_sb = pers.tile([P, C // P, DL], BF16)
    wp_sb = pers.tile([P, 2 * NSEG, DL], BF16)  # seg s = NSEG*p + r
    acc = pers.tile([P, 2, T], BF16)            # proj phase-A accumulator (out^T)

    bqp = pers.tile([P, 2], F32)
    bkp = pers.tile([P, 2], F32)
    bv_row = pers.tile([1, DL], F32)
    bv_bc = pers.tile([P, DL], F32)
    bpp = pers.tile([P, 2], F32)

    # ---- input DMAs: big transfers, ordered so tt=0 matmuls start early ----
    nc.sync.dma_start(wk_sb[:], wk.rearrange("(c p) n -> p c n", p=P))
    nc.sync.dma_start(wq_sb[:], wq.rearrange("(c p) n -> p c n", p=P))
    for tt in range(T // NQ):
        nc.sync.dma_start(
            xt_sb[:, :, ts(tt, NQ)],
            xt[:, ts(tt, NQ)].rearrange("(c p) t -> p c t", p=P),
        )
        if tt == 0:
            nc.sync.dma_start(bqp[:], bq.rearrange("(j p) -> p j", p=P))
            nc.sync.dma_start(bkp[:], bk.rearrange("(j p) -> p j", p=P))
            nc.sync.dma_start(bv_row[:], bv[None, :])
            nc.gpsimd.partition_broadcast(bv_bc[:], bv_row[:])
            nc.sync.dma_start(tri_sb[:], tri)
    nc.sync.dma_start(wv_sb[:], wv.rearrange("(c p) n -> p c n", p=P))
    nc.sync.dma_start(wp_sb[:], wp.rearrange("(s p) n -> p s n", p=P))
    nc.sync.dma_start(bpp[:], bp.rearrange("(o p) -> p o", p=P))

    def qk_pair_tile(j, tt):
        # K and Q chains for one (j, tt), interleaved matmul-by-matmul on
        # two PSUM banks: 16-mm contiguous PE streak (HAM stays warm) and
        # each chain's LDWEIGHTS can overlap the other chain's matmul.
        psk = psum.tile([P, NQ], F32, tag="gemm", name="qk_psk", bufs=2)
        psq = psum.tile([P, NQ], F32, tag="gemm", name="qk_psq", bufs=2)
        for cc in range(C // P):
            mm(
                psk[:],
                wk_sb[:, cc, ts(j, P)],
                xt_sb[:, cc, ts(tt, NQ)],
                start=(cc == 0),
                stop=(cc == C // P - 1),
            )
            mm(
                psq[:],
                wq_sb[:, cc, ts(j, P)],
                xt_sb[:, cc, ts(tt, NQ)],
                start=(cc == 0),
                stop=(cc == C // P - 1),
            )
        nc.vector.tensor_scalar_add(
            kt_sb[:, j, ts(tt, NQ)], psk[:], bkp[:, j : j + 1]
        )
        nc.vector.tensor_scalar_add(
            qt_sb[:, j, ts(tt, NQ)], psq[:], bqp[:, j : j + 1]
        )

    def v_pair_tile(tt):
        # two adjacent V tiles, interleaved on two PSUM banks
        ps0 = psum.tile([P, DL], F32, tag="gemm", name="v_ps0", bufs=2)
        ps1 = psum.tile([P, DL], F32, tag="gemm", name="v_ps1", bufs=2)
        for cc in range(C // P):
            mm(
                ps0[:],
                xt_sb[:, cc, ts(tt, P)],
                wv_sb[:, cc, :],
                start=(cc == 0),
                stop=(cc == C // P - 1),
            )
            mm(
                ps1[:],
                xt_sb[:, cc, ts(tt + 1, P)],
                wv_sb[:, cc, :],
                start=(cc == 0),
                stop=(cc == C // P - 1),
            )
        for k, ps in ((0, ps0), (1, ps1)):
            nc.vector.tensor_add(
                v_sb[:, tt + k, :, 0:D],
                ps[:].rearrange("p (h d) -> p h d", h=HL),
                bv_bc[:].rearrange("p (h d) -> p h d", h=HL),
            )  # cols D..2D keep their memset 1.0 (row-sum columns)

    def attn_qtile(pair, qt):
        q0 = NQ * qt
        nl = q0 // P + NQ // P  # l-chunks for causal coverage
        last_mm = [None]  # last PV matmul of this qtile (for sched ordering)
        o_ps = [
            psum.tile([P, NQ], F32, tag=f"o{hi}", name=f"o_ps{hi}", bufs=1)
            for hi in range(2)
        ]

        def s_stage(lc):
            w0 = max(P * lc - q0, 0)
            s2 = psum.tile([P, 2, NQ], F32, tag="s", name="s2", bufs=2)
            for hi in range(2):
                mm(
                    s2[:, hi, w0:NQ],
                    kt_sb[64 * hi : 64 * hi + 64, pair, ts(lc, P)],
                    qt_sb[64 * hi : 64 * hi + 64, pair, ds(q0 + w0, NQ - w0)],
                    start=True,
                    stop=True,
                    tile_position=(64 * hi, 0),
                )
            return s2

        def pv_stage(lc, s2):
            off = P * lc - q0
            w0 = max(off, 0)
            pt = pp.tile([P, 2, NQ], BF16, tag="p", name="pt")
            nc.scalar.activation(
                pt[:, :, w0:NQ],
                s2[:, :, w0:NQ],
                mybir.ActivationFunctionType.Exp,
                bias=0.0,
                scale=1.0 / np.sqrt(D),
            )
            if off >= 0:
                for hi in range(2):
                    nc.vector.tensor_mul(
                        pt[:, hi, off : off + P],
                        pt[:, hi, off : off + P],
                        tri_sb[:],
                    )
            for hi in range(2):
                last_mm[0] = mm(
                    o_ps[hi][:, w0:NQ],
                    v_sb[:, lc, 2 * pair + hi, :],
                    pt[:, hi, w0:NQ],
                    start=(lc == 0),
                    stop=(lc == nl - 1),
                )  # partitions 0..63 = O, 64..127 = replicated row-sums

        # software pipeline: keep one S stage ahead of exp/PV
        prev = s_stage(0)
        for lc in range(1, nl):
            cur = s_stage(lc)
            pv_stage(lc - 1, prev)
            prev = cur
        pv_stage(nl - 1, prev)

        for hi in range(2):
            # NOTE: reciprocal_approx_fast reading PSUM directly returns
            # garbage (custom-DVE op) -- must bounce through SBUF.
            sums_sb = nrm.tile([D, NQ], F32, tag="sums")
            nc.vector.tensor_copy(sums_sb[:], o_ps[hi][D : 2 * D, :])
            rcp = nrm.tile([D, NQ], F32, tag="rcp")
            nc.vector.reciprocal_approx_fast(rcp[:], sums_sb[:])
            nc.vector.tensor_mul(
                yth[pair][64 * hi : 64 * hi + 64, ds(q0, NQ)],
                o_ps[hi][0:D, :],
                rcp[:],
            )
        return last_mm[0]

    if AG_WORLD == 8:
        replica_groups = [list(range(NC))]
    else:
        replica_groups = [[0, 1, 2, 3], [4, 5, 6, 7]]

    def ship(pair, ci):
        # DMA one column chunk of this pair's Y^T to HBM and AllGather it.
        # gpsimd has no other work, so the trigger blocking it is harmless.
        t0, tw = SHIP_PLAN[pair][ci]
        nc.sync.dma_start(ytl[pair][ci][:], yth[pair][:, ds(t0, tw)])
        nc.gpsimd.collective_compute(
            "AllGather",
            mybir.AluOpType.bypass,
            replica_groups=replica_groups,
            ins=[ytl[pair][ci][:]],
            outs=[ytf[pair][ci][:]],
        )

    def proj_chunk(pair, ci, not_before=None):
        # one pair's contribution to out^T for one shipped column chunk.
        # `not_before` orders the first matmul after the given instruction
        # so the PE stream never head-of-line blocks on the (uncosted) AG.
        t0, tw = SHIP_PLAN[pair][ci]
        y = yf.tile([P, NSEG, tw], BF16, tag=f"y{pair}_{ci}", name=f"y{pair}_{ci}")
        nc.sync.dma_start(
            y[:], ytf[pair][ci].rearrange("(g p) t -> p g t", p=P)
        )
        first = [True]
        for oc in range(2):
            for s in range(tw // NQ):  # 512-col subtiles of the chunk
                tc0 = t0 + s * NQ
                ps = psum.tile([P, NQ], F32, tag="gemm", name="pr_ps", bufs=2)
                for g in range(NSEG):
                    inst = mm(
                        ps[:],
                        wp_sb[:, NSEG * pair + g, ts(oc, P)],
                        y[:, g, ts(s, NQ)],
                        start=(g == 0),
                        stop=(g == NSEG - 1),
                    )
                    if first[0] and not_before is not None:
                        bass._add_dep_helper(
                            inst.ins,
                            not_before.ins,
                            sync=False,
                            reason="proj after attention (AG not costed)",
                        )
                    first[0] = False
                if pair == 0:
                    nc.vector.tensor_scalar_add(
                        acc[:, oc, ds(tc0, NQ)], ps[:], bpp[:, oc : oc + 1]
                    )
                else:
                    ot = po.tile([P, NQ], F32, tag="ot")
                    nc.vector.tensor_add(ot[:], ps[:], acc[:, oc, ds(tc0, NQ)])
                    nc.sync.dma_start(out[ts(oc, P), ds(tc0, NQ)], ot[:])

    # ---------------- program ----------------
    # Phase 1: all of QKV as one dense matmul block (PE stays HAM-warm).
    for tt in range(T // NQ):
        qk_pair_tile(0, tt)
    for tt in range(0, 4, 2):
        v_pair_tile(tt)
    for tt in range(T // NQ):
        qk_pair_tile(1, tt)
    for tt in range(4, T // P, 2):
        v_pair_tile(tt)

    # Phase 2: attention (ACT-bound; ships pipelined out via collectives).
    marks = {}
    for qt in range(T // NQ):
        marks[(0, qt)] = attn_qtile(0, qt)
        if qt == 1:
            ship(0, 0)
    ship(0, 1)
    for qt in range(T // NQ):
        marks[(1, qt)] = attn_qtile(1, qt)
        if qt == 1:
            ship(1, 0)
        if qt == 2:
            ship(1, 1)
    ship(1, 2)

    # Phase 3: output projection. Pair-0 contributions are ordered after
    # early pair-1 attention (their AGs are long done by then) so they
    # fill PE idle during late pair-1 attention and the final AGs; the
    # pair-1 chunks land as their (quarter-sized, floor-bound) AGs finish.
    proj_chunk(0, 0, not_before=marks[(1, 0)])
    proj_chunk(0, 1, not_before=marks[(1, 1)])
    proj_chunk(1, 0, not_before=marks[(1, 2)])
    proj_chunk(1, 1, not_before=marks[(1, 3)])
    proj_chunk(1, 2, not_before=marks[(1, 3)])


def build_program():
    nc = bacc.Bacc(
        "TRN2",
        target_bir_lowering=False,
        debug=False,
        enable_asserts=False,
        num_devices=NC,
    )
    xt = nc.dram_tensor("xt", [C, T], BF16, kind="ExternalInput").ap()
    wq = nc.dram_tensor("wq", [C, DL], BF16, kind="ExternalInput").ap()
    wk = nc.dram_tensor("wk", [C, DL], BF16, kind="ExternalInput").ap()
    wv = nc.dram_tensor("wv", [C, DL], BF16, kind="ExternalInput").ap()
    bq = nc.dram_tensor("bq", [DL], F32, kind="ExternalInput").ap()
    bk = nc.dram_tensor("bk", [DL], F32, kind="ExternalInput").ap()
    bv = nc.dram_tensor("bv", [DL], F32, kind="ExternalInput").ap()
    wp = nc.dram_tensor("wp", [2 * NSEG * P, DL], BF16, kind="ExternalInput").ap()
    bp = nc.dram_tensor("bp", [DL], F32, kind="ExternalInput").ap()
    tri = nc.dram_tensor("tri", [P, P], BF16, kind="ExternalInput").ap()
    out = nc.dram_tensor("out", [DL, T], F32, kind="ExternalOutput").ap()
    ytl = [
        [
            nc.dram_tensor(f"ytl{p}_{ci}", [P, tw], BF16, kind="Internal").ap()
            for ci, (t0, tw) in enumerate(SHIP_PLAN[p])
        ]
        for p in range(2)
    ]
    ytf = [
        [
            nc.dram_tensor(
                f"ytf{p}_{ci}",
                [AG_WORLD * P, tw],
                BF16,
                kind="Internal",
                addr_space="Shared" if AG_WORLD == 8 else "Local",
            ).ap()
            for ci, (t0, tw) in enumerate(SHIP_PLAN[p])
        ]
        for p in range(2)
    ]
    io = (xt, wq, wk, wv, bq, bk, bv, wp, bp, tri, out, ytl, ytf)
    with tile.TileContext(nc) as tc:
        import contextlib

        with contextlib.ExitStack() as ctx:
            _build_body(ctx, tc, io)
    nc.compile()
    return nc


def _stage_wp(W_proj, b, g):
    """wp rows permuted to match ytf row order (rank-major, per pair).

    With AG_WORLD == 8, ytf[p][half] row block r (128 rows) comes from core
    r, carrying batch r//4, group r%4, heads (4*(r%4) + 2p + {0,1}). Blocks
    of the foreign batch get zero weights so one SPMD program serves both
    batches. With AG_WORLD == 4, blocks are the 4 same-batch ranks.
    """
    cols = slice(DL * g, DL * (g + 1))
    Wl = W_proj[:, cols]  # [C, DL]
    segs = []
    for p in range(2):
        if AG_WORLD == 8:
            for r in range(NC):
                if r // NG == b:
                    h0 = 4 * (r % NG) + 2 * p
                    segs.append(Wl[64 * h0 : 64 * h0 + 128, :])
                else:
                    segs.append(np.zeros((P, DL), np.float32))
        else:
            for r in range(NG):
                h0 = 4 * r + 2 * p
                segs.append(Wl[64 * h0 : 64 * h0 + 128, :])
    return np.ascontiguousarray(np.concatenate(segs, axis=0)).astype(NPBF16)


def make_in_maps(x, W_attn, b_attn, W_proj, b_proj):
    # scores are computed transposed (S^T[l, q]); position (l', q'') in a
    # diagonal 128x128 block is causally valid iff q'' >= l' -> upper-tri mask
    tri_np = np.triu(np.ones((P, P), dtype=np.float32)).astype(NPBF16)
    x = np.asarray(x, dtype=np.float32)
    W_attn = np.asarray(W_attn, dtype=np.float32)
    b_attn = np.asarray(b_attn, dtype=np.float32)
    W_proj = np.asarray(W_proj, dtype=np.float32)
    b_proj = np.asarray(b_proj, dtype=np.float32)
    in_maps = []
    for c in range(NC):
        b, g = divmod(c, NG)
        cols = slice(DL * g, DL * (g + 1))
        in_maps.append(
            {
                "xt": np.ascontiguousarray(x[b].T).astype(NPBF16),
                "wq": np.ascontiguousarray(W_attn[:, cols]).astype(NPBF16),
                "wk": np.ascontiguousarray(W_attn[:, C:][:, cols]).astype(NPBF16),
                "wv": np.ascontiguousarray(W_attn[:, 2 * C :][:, cols]).astype(
                    NPBF16
                ),
                "bq": np.ascontiguousarray(b_attn[cols]),
                "bk": np.ascontiguousarray(b_attn[C:][cols]),
                "bv": np.ascontiguousarray(b_attn[2 * C :][cols]),
                "wp": _stage_wp(W_proj, b, g),
                "bp": np.ascontiguousarray(b_proj[cols]),
                "tri": tri_np,
            }
        )
    return in_maps


_NC_CACHE = {}


def _install_ntff_hook():
    """Recreate the missing antenv.axon_hooks module so
    run_bass_kernel_spmd(trace=True) can capture NTFF profiles under axon."""
    import sys
    import types

    if "antenv.axon_hooks" in sys.modules:
        return True
    try:
        from trn_agent_boot.trn_boot import _ntff_profile_via_ctypes

        hook = _ntff_profile_via_ctypes("/opt/axon/libaxon_pjrt.so")
        if hook is None:
            return False
        mod = types.ModuleType("antenv.axon_hooks")
        mod.get_axon_ntff_profile_hook = lambda: hook
        mod.set_axon_ntff_profile_hook = lambda h: None
        sys.modules["antenv.axon_hooks"] = mod
        import antenv

        antenv.axon_hooks = mod
        # keep trace artifacts local (no fish bucket in this container)
        bass_utils.upload_artifacts = lambda tmpdir: tmpdir
        return True
    except Exception:
        return False


def _get_program():
    if "nc" not in _NC_CACHE:
        nc = build_program()
        nc.m = get_hw_module(nc.m)
        _NC_CACHE["nc"] = nc
    return _NC_CACHE["nc"]


def kernel(x, W_attn, b_attn, W_proj, b_proj):
    nc = _get_program()
    in_maps = make_in_maps(x, W_attn, b_attn, W_proj, b_proj)
    trace = bool(int(os.environ.get("KERNEL_TRACE", "0")))
    if trace:
        trace = _install_ntff_hook()
    res = bass_utils.run_bass_kernel_spmd(
        nc,
        in_maps,
        core_ids=list(range(NC)),
        trace=trace,
        trace_cores=list(range(NC)) if trace else None,
    )
    if trace:
        _NC_CACHE["last_results"] = res
        if res.exec_time_ns is not None:
            print(f"HW exec time: {res.exec_time_ns} ns")
            if res.instructions_and_trace is not None:
                print(f"trace: {res.instructions_and_trace[1]}")
    out = np.empty((B, T, C), dtype=np.float32)
    for c in range(NC):
        b, g = divmod(c, NG)
        out[b, :, DL * g : DL * (g + 1)] = res.results[c]["out"].T
    return out


# revision 35
# speedup vs baseline: 1.2113x; 1.1446x over previous
"""Causal self-attention (B=2, T=2048, C=1024, H=16, D=64) on 8 trn2 NeuronCores.

Sharding: batch x head-group. Core c handles batch b = c//4 and head group
g = c%4 (4 heads = 256 channels). All-bf16 data path (fp32 PSUM accumulate).

Per core:
  - warmup matmul burst at t=0 (overlapping input DMA) so the PE HAM clock
    gate flips to 2.4 GHz before real work, and stays there
  - qkv projection for its 4 heads (Q^T/K^T in [d, t] layout, V in [t, d]);
    x^T DMA'd in four 1MB T-slices, weights in single DMAs
  - causal flash attention (scores K-major as S^T, exp batched across both
    heads of a pair in one ACT call from a 2-bank PSUM tile, diagonal
    blocks trimmed, row-sums via a ones-column appended to V)
  - AllGather of Y^T across all 8 cores (the 8-rank on-chip path is much
    faster than a 4-rank ring), split per (pair, T-half) = 4 collectives
    in bf16, pipelined into attention; proj weight rows for the foreign
    batch are staged as zeros so one SPMD program works for both batches
  - output projection in transposed layout (out^T[oc, t], N=512 moving),
    2-phase accumulation interleaved into pair-1 attention

Host gather: per-core out^T [256, T] -> transpose into [B, T, C] slices.
"""

import os
import numpy as np
import ml_dtypes

import concourse.bass as bass
import concourse.bacc as bacc
import concourse.mybir as mybir
import concourse.tile as tile
from concourse import bass_utils
from concourse.bass import ds, ts
from concourse.bass_interp import get_hw_module

P = 128
B, T, C = 2, 2048, 1024
NH, D = 16, 64
NC = 8          # cores
NG = 4          # head groups (cores per batch)
HL = NH // NG   # heads per core = 4
DL = HL * D     # local channels = 256
NQ = 512        # query tile
F32 = mybir.dt.float32
BF16 = mybir.dt.bfloat16
NPBF16 = ml_dtypes.bfloat16

AG_WORLD = 4    # 4: per-batch AllGather groups; 8: one 8-rank AllGather
NSEG = 2 * NG if AG_WORLD == 8 else NG  # proj row-segs per pair
N_WARMUP = 28   # dummy matmuls to warm the HAM clock gate
# Y^T column chunks shipped per pair (start, width). Pair 1's tail is
# quarter-sized so the last (floor-bound) AllGather starts one qtile early.
SHIP_PLAN = {
    0: [(0, T // 2), (T // 2, T // 2)],
    1: [(0, T // 2), (T // 2, T // 4), (3 * T // 4, T // 4)],
}


def _build_body(ctx, tc, io):
    nc = tc.nc
    xt, wq, wk, wv, bq, bk, bv, wp, bp, tri, out, ytl, ytf = io
    mm = nc.tensor.matmul

    pers = ctx.enter_context(tc.tile_pool(name="pers", bufs=1))
    psum = ctx.enter_context(tc.tile_pool(name="psum", bufs=1, space="PSUM"))
    pp = ctx.enter_context(tc.tile_pool(name="pp", bufs=3))
    nrm = ctx.enter_context(tc.tile_pool(name="nrm", bufs=4))
    po = ctx.enter_context(tc.tile_pool(name="po", bufs=4))
    yf = ctx.enter_context(tc.tile_pool(name="yf", bufs=2))

    # ---- HAM warmup: dense dummy matmuls while input DMAs stream in ----
    dummy = pers.tile([P, NQ], BF16)
    nc.vector.memset(dummy[:], 0.0)
    for _ in range(N_WARMUP):
        wps = psum.tile([P, NQ], F32, tag="gemm", name="warm_ps", bufs=2)
        mm(wps[:], dummy[:, 0:P], dummy[:], start=True, stop=True)

    tri_sb = pers.tile([P, P], BF16)
    qt_sb = pers.tile([P, 2, T], BF16)   # pair j; head 2j+1 on partitions 64..127
    kt_sb = pers.tile([P, 2, T], BF16)
    # [l_part, l_chunk, head, d | 64x ones]: PV with this stationary yields
    # O on partitions 0..63 and the softmax row-sum REPLICATED on 64..127,
    # so normalization needs no cross-partition broadcast at all.
    v_sb = pers.tile([P, T // P, HL, 2 * D], BF16)
    nc.vector.memset(v_sb[:], 1.0)  # cols D..2D stay 1; 0..D overwritten
    # yth[pair]: rows 0..63 head 2p, rows 64..127 head 2p+1 (AG payload layout)
    yth = [pers.tile([P, T], BF16, tag=f"yth{p}", name=f"yth{p}") for p in range(2)]

    xt_sb = pers.tile([P, C // P, T], BF16)
    wq_sb = pers.tile([P, C // P, DL], BF16)
    wk_sb = pers.tile([P, C // P, DL], BF16)
    wv_sb = pers.tile([P, C // P, DL], BF16)
    wp_sb = pers.tile([P, 2 * NSEG, DL], BF16)  # seg s = NSEG*p + r
    acc = pers.tile([P, 2, T], BF16)            # proj phase-A accumulator (out^T)

    bqp = pers.tile([P, 2], F32)
    bkp = pers.tile([P, 2], F32)
    bv_row = pers.tile([1, DL], F32)
    bv_bc = pers.tile([P, DL], F32)
    bpp = pers.tile([P, 2], F32)

    # ---- input DMAs: big transfers, ordered so tt=0 matmuls start early ----
    nc.sync.dma_start(wk_sb[:], wk.rearrange("(c p) n -> p c n", p=P))
    nc.sync.dma_start(wq_sb[:], wq.rearrange("(c p) n -> p c n", p=P))
    for tt in range(T // NQ):
        nc.sync.dma_start(
            xt_sb[:, :, ts(tt, NQ)],
            xt[:, ts(tt, NQ)].rearrange("(c p) t -> p c t", p=P),
        )
        if tt == 0:
            nc.sync.dma_start(bqp[:], bq.rearrange("(j p) -> p j", p=P))
            nc.sync.dma_start(bkp[:], bk.rearrange("(j p) -> p j", p=P))
            nc.sync.dma_start(bv_row[:], bv[None, :])
            nc.gpsimd.partition_broadcast(bv_bc[:], bv_row[:])
            nc.sync.dma_start(tri_sb[:], tri)
    nc.sync.dma_start(wv_sb[:], wv.rearrange("(c p) n -> p c n", p=P))
    nc.sync.dma_start(wp_sb[:], wp.rearrange("(s p) n -> p s n", p=P))
    nc.sync.dma_start(bpp[:], bp.rearrange("(o p) -> p o", p=P))

    def qk_pair_tile(j, tt):
        # K and Q chains for one (j, tt), interleaved matmul-by-matmul on
        # two PSUM banks: 16-mm contiguous PE streak (HAM stays warm) and
        # each chain's LDWEIGHTS can overlap the other chain's matmul.
        psk = psum.tile([P, NQ], F32, tag="gemm", name="qk_psk", bufs=2)
        psq = psum.tile([P, NQ], F32, tag="gemm", name="qk_psq", bufs=2)
        for cc in range(C // P):
            mm(
                psk[:],
                wk_sb[:, cc, ts(j, P)],
                xt_sb[:, cc, ts(tt, NQ)],
                start=(cc == 0),
                stop=(cc == C // P - 1),
            )
            mm(
                psq[:],
                wq_sb[:, cc, ts(j, P)],
                xt_sb[:, cc, ts(tt, NQ)],
                start=(cc == 0),
                stop=(cc == C // P - 1),
            )
        nc.vector.tensor_scalar_add(
            kt_sb[:, j, ts(tt, NQ)], psk[:], bkp[:, j : j + 1]
        )
        nc.vector.tensor_scalar_add(
            qt_sb[:, j, ts(tt, NQ)], psq[:], bqp[:, j : j + 1]
        )

    def v_pair_tile(tt):
        # two adjacent V tiles, interleaved on two PSUM banks
        ps0 = psum.tile([P, DL], F32, tag="gemm", name="v_ps0", bufs=2)
        ps1 = psum.tile([P, DL], F32, tag="gemm", name="v_ps1", bufs=2)
        for cc in range(C // P):
            mm(
                ps0[:],
                xt_sb[:, cc, ts(tt, P)],
                wv_sb[:, cc, :],
                start=(cc == 0),
                stop=(cc == C // P - 1),
            )
            mm(
                ps1[:],
                xt_sb[:, cc, ts(tt + 1, P)],
                wv_sb[:, cc, :],
                start=(cc == 0),
                stop=(cc == C // P - 1),
            )
        for k, ps in ((0, ps0), (1, ps1)):
            nc.vector.tensor_add(
                v_sb[:, tt + k, :, 0:D],
                ps[:].rearrange("p (h d) -> p h d", h=HL),
                bv_bc[:].rearrange("p (h d) -> p h d", h=HL),
            )  # cols D..2D keep their memset 1.0 (row-sum columns)

    def attn_qtile(pair, qt):
        q0 = NQ * qt
        nl = q0 // P + NQ // P  # l-chunks for causal coverage
        last_mm = [None]  # last PV matmul of this qtile (for sched ordering)
        o_ps = [
            psum.tile([P, NQ], F32, tag=f"o{hi}", name=f"o_ps{hi}", bufs=1)
            for hi in range(2)
        ]

        def s_stage(lc):
            w0 = max(P * lc - q0, 0)
            s2 = psum.tile([P, 2, NQ], F32, tag="s", name="s2", bufs=2)
            for hi in range(2):
                mm(
                    s2[:, hi, w0:NQ],
                    kt_sb[64 * hi : 64 * hi + 64, pair, ts(lc, P)],
                    qt_sb[64 * hi : 64 * hi + 64, pair, ds(q0 + w0, NQ - w0)],
                    start=True,
                    stop=True,
                    tile_position=(64 * hi, 0),
                )
            return s2

        def pv_stage(lc, s2):
            off = P * lc - q0
            w0 = max(off, 0)
            pt = pp.tile([P, 2, NQ], BF16, tag="p", name="pt")
            nc.scalar.activation(
                pt[:, :, w0:NQ],
                s2[:, :, w0:NQ],
                mybir.ActivationFunctionType.Exp,
                bias=0.0,
                scale=1.0 / np.sqrt(D),
            )
            if off >= 0:
                for hi in range(2):
                    nc.vector.tensor_mul(
                        pt[:, hi, off : off + P],
                        pt[:, hi, off : off + P],
                        tri_sb[:],
                    )
            for hi in range(2):
                last_mm[0] = mm(
                    o_ps[hi][:, w0:NQ],
                    v_sb[:, lc, 2 * pair + hi, :],
                    pt[:, hi, w0:NQ],
                    start=(lc == 0),
                    stop=(lc == nl - 1),
                )  # partitions 0..63 = O, 64..127 = replicated row-sums

        # software pipeline: keep one S stage ahead of exp/PV
        prev = s_stage(0)
        for lc in range(1, nl):
            cur = s_stage(lc)
            pv_stage(lc - 1, prev)
            prev = cur
        pv_stage(nl - 1, prev)

        for hi in range(2):
            # NOTE: reciprocal_approx_fast reading PSUM directly returns
            # garbage (custom-DVE op) -- must bounce through SBUF.
            sums_sb = nrm.tile([D, NQ], F32, tag="sums")
            nc.vector.tensor_copy(sums_sb[:], o_ps[hi][D : 2 * D, :])
            rcp = nrm.tile([D, NQ], F32, tag="rcp")
            nc.vector.reciprocal_approx_fast(rcp[:], sums_sb[:])
            nc.vector.tensor_mul(
                yth[pair][64 * hi : 64 * hi + 64, ds(q0, NQ)],
                o_ps[hi][0:D, :],
                rcp[:],
            )
        return last_mm[0]

    if AG_WORLD == 8:
        replica_groups = [list(range(NC))]
    else:
        replica_groups = [[0, 1, 2, 3], [4, 5, 6, 7]]

    def ship(pair, ci):
        # DMA one column chunk of this pair's Y^T to HBM and AllGather it.
        # gpsimd has no other work, so the trigger blocking it is harmless.
        t0, tw = SHIP_PLAN[pair][ci]
        nc.sync.dma_start(ytl[pair][ci][:], yth[pair][:, ds(t0, tw)])
        nc.gpsimd.collective_compute(
            "AllGather",
            mybir.AluOpType.bypass,
            replica_groups=replica_groups,
            ins=[ytl[pair][ci][:]],
            outs=[ytf[pair][ci][:]],
        )

    def proj_chunk(pair, ci, not_before=None):
        # one pair's contribution to out^T for one shipped column chunk.
        # `not_before` orders the first matmul after the given instruction
        # so the PE stream never head-of-line blocks on the (uncosted) AG.
        t0, tw = SHIP_PLAN[pair][ci]

        def order(inst):
            if not_before is not None:
                bass._add_dep_helper(
                    inst.ins,
                    not_before.ins,
                    sync=False,
                    reason="proj after attention (AG not costed)",
                )
            return inst

        y = yf.tile([P, NSEG, tw], BF16, tag=f"y{pair}_{ci}", name=f"y{pair}_{ci}")
        order(
            nc.sync.dma_start(
                y[:], ytf[pair][ci].rearrange("(g p) t -> p g t", p=P)
            )
        )
        for oc in range(2):
            for s in range(tw // NQ):  # 512-col subtiles of the chunk
                tc0 = t0 + s * NQ
                ps = psum.tile([P, NQ], F32, tag="gemm", name="pr_ps", bufs=2)
                for g in range(NSEG):
                    order(
                        mm(
                            ps[:],
                            wp_sb[:, NSEG * pair + g, ts(oc, P)],
                            y[:, g, ts(s, NQ)],
                            start=(g == 0),
                            stop=(g == NSEG - 1),
                        )
                    )
                if pair == 0:
                    order(
                        nc.vector.tensor_scalar_add(
                            acc[:, oc, ds(tc0, NQ)], ps[:], bpp[:, oc : oc + 1]
                        )
                    )
                else:
                    ot = po.tile([P, NQ], F32, tag="ot")
                    order(
                        nc.vector.tensor_add(
                            ot[:], ps[:], acc[:, oc, ds(tc0, NQ)]
                        )
                    )
                    order(nc.sync.dma_start(out[ts(oc, P), ds(tc0, NQ)], ot[:]))

    # ---------------- program ----------------
    # Phase 1: all of QKV as one dense matmul block (PE stays HAM-warm).
    for tt in range(T // NQ):
        qk_pair_tile(0, tt)
    for tt in range(0, 4, 2):
        v_pair_tile(tt)
    for tt in range(T // NQ):
        qk_pair_tile(1, tt)
    for tt in range(4, T // P, 2):
        v_pair_tile(tt)

    # Phase 2: attention (ACT-bound; ships pipelined out via collectives).
    marks = {}
    for qt in range(T // NQ):
        marks[(0, qt)] = attn_qtile(0, qt)
        if qt == 1:
            ship(0, 0)
    ship(0, 1)
    for qt in range(T // NQ):
        marks[(1, qt)] = attn_qtile(1, qt)
        if qt == 1:
            ship(1, 0)
        if qt == 2:
            ship(1, 1)
    ship(1, 2)

    # Phase 3: output projection. Pair-0 contributions are ordered after
    # early pair-1 attention (their AGs are long done by then) so they
    # fill PE idle during late pair-1 attention and the final AGs; the
    # pair-1 chunks land as their (quarter-sized, floor-bound) AGs finish.
    proj_chunk(0, 0, not_before=marks[(1, 0)])
    proj_chunk(0, 1, not_before=marks[(1, 1)])
    proj_chunk(1, 0, not_before=marks[(1, 2)])
    proj_chunk(1, 1, not_before=marks[(1, 3)])
    proj_chunk(1, 2, not_before=marks[(1, 3)])


def build_program():
    nc = bacc.Bacc(
        "TRN2",
        target_bir_lowering=False,
        debug=False,
        enable_asserts=False,
        num_devices=NC,
    )
    xt = nc.dram_tensor("xt", [C, T], BF16, kind="ExternalInput").ap()
    wq = nc.dram_tensor("wq", [C, DL], BF16, kind="ExternalInput").ap()
    wk = nc.dram_tensor("wk", [C, DL], BF16, kind="ExternalInput").ap()
    wv = nc.dram_tensor("wv", [C, DL], BF16, kind="ExternalInput").ap()
    bq = nc.dram_tensor("bq", [DL], F32, kind="ExternalInput").ap()
    bk = nc.dram_tensor("bk", [DL], F32, kind="ExternalInput").ap()
    bv = nc.dram_tensor("bv", [DL], F32, kind="ExternalInput").ap()
    wp = nc.dram_tensor("wp", [2 * NSEG * P, DL], BF16, kind="ExternalInput").ap()
    bp = nc.dram_tensor("bp", [DL], F32, kind="ExternalInput").ap()
    tri = nc.dram_tensor("tri", [P, P], BF16, kind="ExternalInput").ap()
    out = nc.dram_tensor("out", [DL, T], F32, kind="ExternalOutput").ap()
    ytl = [
        [
            nc.dram_tensor(f"ytl{p}_{ci}", [P, tw], BF16, kind="Internal").ap()
            for ci, (t0, tw) in enumerate(SHIP_PLAN[p])
        ]
        for p in range(2)
    ]
    ytf = [
        [
            nc.dram_tensor(
                f"ytf{p}_{ci}",
                [AG_WORLD * P, tw],
                BF16,
                kind="Internal",
                addr_space="Shared" if AG_WORLD == 8 else "Local",
            ).ap()
            for ci, (t0, tw) in enumerate(SHIP_PLAN[p])
        ]
        for p in range(2)
    ]
    io = (xt, wq, wk, wv, bq, bk, bv, wp, bp, tri, out, ytl, ytf)
    with tile.TileContext(nc) as tc:
        import contextlib

        with contextlib.ExitStack() as ctx:
            _build_body(ctx, tc, io)
    nc.compile()
    return nc


def _stage_wp(W_proj, b, g):
    """wp rows permuted to match ytf row order (rank-major, per pair).

    With AG_WORLD == 8, ytf[p][half] row block r (128 rows) comes from core
    r, carrying batch r//4, group r%4, heads (4*(r%4) + 2p + {0,1}). Blocks
    of the foreign batch get zero weights so one SPMD program serves both
    batches. With AG_WORLD == 4, blocks are the 4 same-batch ranks.
    """
    cols = slice(DL * g, DL * (g + 1))
    Wl = W_proj[:, cols]  # [C, DL]
    segs = []
    for p in range(2):
        if AG_WORLD == 8:
            for r in range(NC):
                if r // NG == b:
                    h0 = 4 * (r % NG) + 2 * p
                    segs.append(Wl[64 * h0 : 64 * h0 + 128, :])
                else:
                    segs.append(np.zeros((P, DL), np.float32))
        else:
            for r in range(NG):
                h0 = 4 * r + 2 * p
                segs.append(Wl[64 * h0 : 64 * h0 + 128, :])
    return np.ascontiguousarray(np.concatenate(segs, axis=0)).astype(NPBF16)


def make_in_maps(x, W_attn, b_attn, W_proj, b_proj):
    # scores are computed transposed (S^T[l, q]); position (l', q'') in a
    # diagonal 128x128 block is causally valid iff q'' >= l' -> upper-tri mask
    tri_np = np.triu(np.ones((P, P), dtype=np.float32)).astype(NPBF16)
    x = np.asarray(x, dtype=np.float32)
    W_attn = np.asarray(W_attn, dtype=np.float32)
    b_attn = np.asarray(b_attn, dtype=np.float32)
    W_proj = np.asarray(W_proj, dtype=np.float32)
    b_proj = np.asarray(b_proj, dtype=np.float32)
    in_maps = []
    for c in range(NC):
        b, g = divmod(c, NG)
        cols = slice(DL * g, DL * (g + 1))
        in_maps.append(
            {
                "xt": np.ascontiguousarray(x[b].T).astype(NPBF16),
                "wq": np.ascontiguousarray(W_attn[:, cols]).astype(NPBF16),
                "wk": np.ascontiguousarray(W_attn[:, C:][:, cols]).astype(NPBF16),
                "wv": np.ascontiguousarray(W_attn[:, 2 * C :][:, cols]).astype(
                    NPBF16
                ),
                "bq": np.ascontiguousarray(b_attn[cols]),
                "bk": np.ascontiguousarray(b_attn[C:][cols]),
                "bv": np.ascontiguousarray(b_attn[2 * C :][cols]),
                "wp": _stage_wp(W_proj, b, g),
                "bp": np.ascontiguousarray(b_proj[cols]),
                "tri": tri_np,
            }
        )
    return in_maps


_NC_CACHE = {}


def _install_ntff_hook():
    """Recreate the missing antenv.axon_hooks module so
    run_bass_kernel_spmd(trace=True) can capture NTFF profiles under axon."""
    import sys
    import types

    if "antenv.axon_hooks" in sys.modules:
        return True
    try:
        from trn_agent_boot.trn_boot import _ntff_profile_via_ctypes

        hook = _ntff_profile_via_ctypes("/opt/axon/libaxon_pjrt.so")
        if hook is None:
            return False
        mod = types.ModuleType("antenv.axon_hooks")
        mod.get_axon_ntff_profile_hook = lambda: hook
        mod.set_axon_ntff_profile_hook = lambda h: None
        sys.modules["antenv.axon_hooks"] = mod
        import antenv

        antenv.axon_hooks = mod
        # keep trace artifacts local (no fish bucket in this container)
        bass_utils.upload_artifacts = lambda tmpdir: tmpdir
        return True
    except Exception:
        return False


def _get_program():
    if "nc" not in _NC_CACHE:
        nc = build_program()
        nc.m = get_hw_module(nc.m)
        _NC_CACHE["nc"] = nc
    return _NC_CACHE["nc"]


def kernel(x, W_attn, b_attn, W_proj, b_proj):
    nc = _get_program()
    in_maps = make_in_maps(x, W_attn, b_attn, W_proj, b_proj)
    trace = bool(int(os.environ.get("KERNEL_TRACE", "0")))
    if trace:
        trace = _install_ntff_hook()
    res = bass_utils.run_bass_kernel_spmd(
        nc,
        in_maps,
        core_ids=list(range(NC)),
        trace=trace,
        trace_cores=list(range(NC)) if trace else None,
    )
    if trace:
        _NC_CACHE["last_results"] = res
        if res.exec_time_ns is not None:
            print(f"HW exec time: {res.exec_time_ns} ns")
            if res.instructions_and_trace is not None:
                print(f"trace: {res.instructions_and_trace[1]}")
    out = np.empty((B, T, C), dtype=np.float32)
    for c in range(NC):
        b, g = divmod(c, NG)
        out[b, :, DL * g : DL * (g + 1)] = res.results[c]["out"].T
    return out
